# revision 31
# baseline (speedup 1.0000x reference)
"""Trainium2 Bass kernel for nn_Attention_25572235280790.

Dense attention block (B=16, C=256, H=W=32, NH=8, HD=32) with RoPE-style theta
shift, LePE depthwise 5x5 conv, BN+SiLU gate, channel LayerNorms and 1x1 convs.

Sharding: data-parallel over batch across 8 NeuronCores (2 batches/core), no
collectives.  Inside each core everything is computed in two layouts:
  - [c, l]  (channels on partitions)  for the 1x1 convs / scores / lepe
  - [l, c]  (spatial on partitions)   for softmax-normalize / layernorms / gate

v4 schedule (255.7us -> 228.5us): PV runs as INCREMENTAL accumulation chains
inside each head-group's own scores phase, lagging one mt-group behind the
exp stream, so no PV lump ever trails the scores.  Chains share PSUM banks
without per-chain start=True (a start lazily zeroes the whole 2KB region,
wiping sibling chains' first contribution): only each bank's first matmul
starts; every other chain's first write replaces via the pending-zero flags.
Each l-tile half's LN1/LN2/proj tail is split into per-lt units pinned one
per scores-jp in the NEXT phase; batch-0's endgame runs as fillers inside
batch-1's first phase.  The trailing half's tails split into column halves:
the left half (hg0 attn + lepe ct0) runs early in the last phase with
partial bn_stats, aggregated against the late right half via 2-block
bn_aggr.  Startup DMAs are chunked so the first conv's exact inputs land
first, striped over the sync/gpsimd/scalar queues.  Scores S^T[m, l] for 4
heads are concurrent row-tiled matmuls (tile_position=(32i,0), K=32, bus
limit 2 col-transfers/cycle) into 2-bank PSUM pairs evacuated by merged exp
ACTs (N=1024) on ScalarE.  LN rsqrt is quake-seed + 1 Newton step on
VectorE so ScalarE never leaves the exp/tanh table set; the final tail's
PSUM evacuations and copies run on ScalarE (idle there), never GpSimd
(no PSUM access, ~4x slower tensor ops).
"""

import numpy as np
import ml_dtypes

import concourse.bass as bass
import concourse.tile as tile
from concourse import bacc
from concourse import mybir
from concourse.alu_op_type import AluOpType

B, C, H, W = 16, 256, 32, 32
NH, HD = 8, 32
SCALE = HD ** -0.5
LN_EPS = 1e-6
BN_EPS = 1e-5
L = H * W
NCORES = 8
BPC = B // NCORES          # batches per core
AF = mybir.ActivationFunctionType
F32 = mybir.dt.float32
BF16 = mybir.dt.bfloat16
I32 = mybir.dt.int32

NPBF = ml_dtypes.bfloat16
# es tiles are fp8-e3m4: softmax is shift-invariant, so exp(s*SCALE - 0.5)
# keeps the observed range [0.057, 20.9] inside e3m4's [~0.008, 15.5] while
# e3m4's 4 mantissa bits halve the quantization error of e4m3.
ES_DT = mybir.dt.float8e3
ES_SHIFT = -0.5
ES_BUFS = 10


def build_program():
    nc = bacc.Bacc()
    dp = nc.declare_dram_parameter
    io = {
        'x2':      dp('x2',      [BPC, C, L],   BF16, isOutput=False),
        'wqkT':    dp('wqkT',    [C, 512],      BF16, isOutput=False),
        'bqk':     dp('bqk',     [128, 4],      F32,  isOutput=False),
        'wvT':     dp('wvT',     [C, C],        BF16, isOutput=False),
        'bv':      dp('bv',      [128, 2],      F32,  isOutput=False),
        'rhsvg':   dp('rhsvg',   [C, 512],      BF16, isOutput=False),
        'bvgbc':   dp('bvgbc',   [128, 512],    BF16, isOutput=False),
        'wprojT':  dp('wprojT',  [C, C],        BF16, isOutput=False),
        'bproj':   dp('bproj',   [128, 2],      F32,  isOutput=False),
        'cosq':    dp('cosq',    [128, L],      BF16, isOutput=False),
        'sinq':    dp('sinq',    [128, L],      BF16, isOutput=False),
        'rotmat':  dp('rotmat',  [128, 128],    BF16, isOutput=False),
        'ident':   dp('ident',   [128, 128],    BF16, isOutput=False),
        'diagw':   dp('diagw',   [2, 128, 25 * 128], BF16, isOutput=False),
        'blepe':   dp('blepe',   [128, 2],      F32,  isOutput=False),
        'g1bc':    dp('g1bc',    [128, C],      F32,  isOutput=False),
        'b1bc':    dp('b1bc',    [128, C],      F32,  isOutput=False),
        'out':     dp('out',     [BPC, C, L],   F32,  isOutput=True),
    }
    with tile.TileContext(nc) as tc:
        _emit(tc, io)
    nc.compile()
    return nc


def _emit(tc, io):
    with (tc.tile_pool(name="cw", bufs=1) as cw,
          tc.tile_pool(name="sb", bufs=2) as sb,
          tc.tile_pool(name="pp", bufs=2, space="PSUM") as pp):
        _emit_body(tc, io, cw, sb, pp)


def _emit_body(tc, io, cw, sb, pp):
    nc = tc.nc

    # ------------------------------------------------------------------
    # persistent constants -- startup DMAs fan out across engine queues
    # (each engine's sequencer owns its own DMA ring; Pool issue is cheap)
    # ------------------------------------------------------------------
    dma_engs = [nc.sync, nc.gpsimd, nc.scalar]
    _dma_rr = [0]

    def dma(out, in_, eng=None):
        e = dma_engs[_dma_rr[0] % len(dma_engs)] if eng is None else eng
        _dma_rr[0] += 1
        e.dma_start(out=out, in_=in_)

    def cload(name, dtype, eng=None):
        src = io[name]
        t = cw.tile(list(src.shape), dtype, name=f"c_{name}")
        dma(out=t, in_=src[:], eng=eng)
        return t

    def cload2(name, dtype, cols, engs=(None, None)):
        ts = [cw.tile([128, cols], dtype, name=f"c_{name}{i}") for i in range(2)]
        for i in range(2):
            dma(out=ts[i], in_=io[name][i * 128:(i + 1) * 128, :], eng=engs[i])
        return ts

    # first wave, chunked so the first qk conv's exact inputs (wqkT m-cols
    # 0:128 + 256:384, x l-cols 0:512) land first, striped over all 3 queues
    wqkT = [cw.tile([128, 512], BF16, name=f"c_wqkT{i}") for i in range(2)]
    xt_a = {0: [], 1: []}
    for b_ in (0,):
        for ct_ in range(2):
            xt_a[b_].append(sb.tile([128, L], BF16, name=f"x_b{b_}c{ct_}",
                                    tag="xt", bufs=4))
    q3 = (nc.sync, nc.gpsimd, nc.scalar)
    first_wave = [
        (wqkT[0][:, 0:128],   io['wqkT'][0:128, 0:128]),
        (wqkT[1][:, 0:128],   io['wqkT'][128:256, 0:128]),
        (wqkT[0][:, 256:384], io['wqkT'][0:128, 256:384]),
        (wqkT[1][:, 256:384], io['wqkT'][128:256, 256:384]),
        (xt_a[0][0][:, 0:512], io['x2'][0, 0:128, 0:512]),
        (xt_a[0][1][:, 0:512], io['x2'][0, 128:256, 0:512]),
        (xt_a[0][0][:, 512:L], io['x2'][0, 0:128, 512:L]),
        (xt_a[0][1][:, 512:L], io['x2'][0, 128:256, 512:L]),
        (wqkT[0][:, 128:256], io['wqkT'][0:128, 128:256]),
        (wqkT[1][:, 128:256], io['wqkT'][128:256, 128:256]),
        (wqkT[0][:, 384:512], io['wqkT'][0:128, 384:512]),
        (wqkT[1][:, 384:512], io['wqkT'][128:256, 384:512]),
    ]
    for j, (dst, src) in enumerate(first_wave[:8]):
        dma(out=dst, in_=src, eng=q3[j % 3])

    def load_x(b_, engs):
        for ct_ in range(2):
            x_t = sb.tile([128, L], BF16, name=f"x_b{b_}c{ct_}", tag="xt",
                          bufs=4)
            dma(out=x_t, in_=io['x2'][b_, ct_ * 128:(ct_ + 1) * 128, :],
                eng=engs[ct_])
            xt_a[b_].append(x_t)
    bqk = cload('bqk', F32, eng=nc.gpsimd)
    rotmat = cload('rotmat', BF16, eng=nc.scalar)
    # rope's first-half tables + rhsvg (gates vg at jp0) land before the
    # deferred wqkT b-chunks (qk fillers at jp1-2) and everything else
    cosq = cw.tile([128, L], BF16, name="c_cosq")
    sinq = cw.tile([128, L], BF16, name="c_sinq")
    dma(out=cosq[:, 0:512], in_=io['cosq'][:, 0:512], eng=nc.gpsimd)
    dma(out=sinq[:, 0:512], in_=io['sinq'][:, 0:512], eng=nc.sync)
    rhsvg = cload2('rhsvg', BF16, 512, engs=(nc.scalar, nc.sync))
    for j, (dst, src) in enumerate(first_wave[8:]):
        dma(out=dst, in_=src, eng=q3[(j + 1) % 3])
    dma(out=cosq[:, 512:L], in_=io['cosq'][:, 512:L], eng=nc.scalar)
    dma(out=sinq[:, 512:L], in_=io['sinq'][:, 512:L], eng=nc.gpsimd)
    bvgbc = cload('bvgbc', BF16, eng=nc.gpsimd)
    wvT = cload2('wvT', BF16, C, engs=(nc.scalar, nc.gpsimd))
    bv = cload('bv', F32)
    ident = cload('ident', BF16)
    blepe = cload('blepe', F32)
    g1bc = cload('g1bc', F32)
    b1bc = cload('b1bc', F32)
    wprojT = cload2('wprojT', BF16, C)
    bproj = cload('bproj', F32)
    load_x(1, (nc.sync, nc.gpsimd))

    epsc = cw.tile([128, 1], F32, name="epsc")
    nc.gpsimd.memset(epsc, LN_EPS)
    esshift = cw.tile([128, 1], F32, name="esshift")
    nc.gpsimd.memset(esshift, ES_SHIFT)
    # dummy exp as the first ScalarE instruction: pulls the ~1.3us ACT
    # table load into the DMA window instead of the first evacuation
    warmup = cw.tile([128, 1], F32, name="warmup")
    nc.scalar.activation(out=warmup, in_=epsc, func=AF.Exp)

    # lepe diagonal stationaries diag[ct][:, tap, :] = diag(w5[ct][:, tap])
    diag = [cw.tile([128, 25, 128], BF16, name=f"diag{i}") for i in range(2)]
    for i in range(2):
        dma(out=diag[i].rearrange("p a b -> p (a b)"), in_=io['diagw'][i],
            eng=(nc.sync, nc.gpsimd)[i])

    qk_a, vT_a, gate_a, vpad_a, lepe_a, y_a = ({} for _ in range(6))
    es_a = {}

    def emit_qk_conv(b, m, scalar_evac=False, big_ps=False):
        # one M-tile of the q/k 1x1 conv; 4 M-tiles: q0 q1 k0 k1.
        # big_ps: pre-phase calls borrow the (idle) ps2 ring so the
        # single-buffered ps_mm tag doesn't serialize the startup chain.
        if b not in qk_a:
            qk_a[b] = {}
        qk_t = sb.tile([128, L], BF16, name=f"qk_b{b}m{m}", tag="qk", bufs=8)
        for n in range(2):
            if big_ps:
                ps = pp.tile([128, 512], F32, name="ps_mmb", tag="ps2", bufs=2)
            else:
                ps = pp.tile([128, 512], F32, name="ps_mm", tag="ps_mm", bufs=1)
            for kc in range(2):
                nc.tensor.matmul(
                    ps, wqkT[kc][:, m * 128:(m + 1) * 128],
                    xt_a[b][kc][:, n * 512:(n + 1) * 512],
                    start=(kc == 0), stop=(kc == 1))
            if scalar_evac:
                nc.scalar.activation(
                    out=qk_t[:, n * 512:(n + 1) * 512], in_=ps,
                    func=AF.Identity, bias=bqk[:, m:m + 1], scale=1.0)
            else:
                nc.vector.tensor_scalar_add(
                    out=qk_t[:, n * 512:(n + 1) * 512], in0=ps,
                    scalar1=bqk[:, m:m + 1])
        qk_a[b][m] = qk_t

    def emit_rope(b, t, dve_mul=False, big_ps=False, halves=(0, 1)):
        # theta shift on q/k tile t, in [d, l] layout.
        for n in halves:
            sl = slice(n * 512, (n + 1) * 512)
            if big_ps:
                ps = pp.tile([128, 512], F32, name="ps_rotb", tag="ps2", bufs=2)
            else:
                ps = pp.tile([128, 512], F32, name="ps_rot", tag="ps_mm", bufs=1)
            nc.tensor.matmul(ps, rotmat, qk_a[b][t][:, sl],
                             start=True, stop=True)
            gtmp = sb.tile([128, 512], BF16, name="rope_g", tag="rope_g", bufs=2)
            mul_eng = nc.vector if dve_mul else nc.gpsimd
            mul_eng.tensor_mul(out=gtmp, in0=qk_a[b][t][:, sl],
                               in1=cosq[:, sl])
            vtmp = sb.tile([128, 512], BF16, name="rope_v", tag="rope_v", bufs=2)
            nc.vector.tensor_mul(out=vtmp, in0=ps, in1=sinq[:, sl])
            nc.vector.tensor_add(out=qk_a[b][t][:, sl], in0=gtmp, in1=vtmp)

    def emit_vg(b, lt):
        # x-stationary pass: v^T and gate^T in [l, .] layout.
        # gate = g*(1+tanh(g/2)) = 2*silu(g); tanh shares the Exp ACT table.
        if lt == 0:
            vT_a[b] = []
            gate_a[b] = []
        ps = pp.tile([128, 512], F32, name="ps_vg", tag="ps_mm", bufs=1)
        for kc in range(2):
            nc.tensor.matmul(
                ps, xt_a[b][kc][:, lt * 128:(lt + 1) * 128], rhsvg[kc],
                start=(kc == 0), stop=(kc == 1))
        vT_t = sb.tile([128, NH, HD + 1], BF16, name=f"vT_b{b}l{lt}",
                       tag="vT", bufs=16)
        nc.gpsimd.memset(vT_t[:, :, HD:HD + 1], 1.0)
        nc.vector.tensor_tensor(
            out=vT_t[:, :, 0:HD],
            in0=ps[:, 0:256].rearrange("p (h d) -> p h d", h=NH),
            in1=bvgbc[:, 0:256].rearrange("p (h d) -> p h d", h=NH),
            op=AluOpType.add)
        vT_a[b].append(vT_t)
        gate_t = sb.tile([128, C], BF16, name=f"gate_b{b}l{lt}",
                         tag="gate_t", bufs=3)
        gb = sb.tile([128, C], F32, name="gb", tag="gb", bufs=4)
        nc.vector.tensor_add(out=gb, in0=ps[:, 256:512], in1=bvgbc[:, 256:512])
        tnt = sb.tile([128, C], F32, name="tnt", tag="tnt", bufs=3)
        nc.scalar.activation(out=tnt, in_=gb, func=AF.Tanh, scale=0.5)
        wt_ = sb.tile([128, C], F32, name="wt_", tag="wt_", bufs=2)
        nc.gpsimd.tensor_mul(out=wt_, in0=gb, in1=tnt)
        nc.gpsimd.tensor_add(out=gate_t, in0=wt_, in1=gb)
        # fold the LN1 affine into the gate here (off the tail critical
        # path): y_ln1 = ((y-mu)*rs)*gate2 + bg2
        gate2_t = sb.tile([128, C], BF16, name=f"gate2_b{b}l{lt}",
                          tag="gate", bufs=16)
        nc.gpsimd.tensor_mul(out=gate2_t, in0=gate_t, in1=g1bc)
        bg2_t = sb.tile([128, C], BF16, name=f"bg2_b{b}l{lt}",
                        tag="bg2", bufs=16)
        nc.gpsimd.tensor_mul(out=bg2_t, in0=gate_t, in1=b1bc)
        gate_a[b].append((gate2_t, bg2_t))

    def emit_vcl(b, ct, n):
        # v in [c, l] (for lepe), into zero-padded image tiles
        if ct == 0 and n == 0:
            vpad = []
            for c2 in range(2):
                vp = sb.tile([128, 36, 36], BF16, name=f"vpad_b{b}c{c2}",
                             tag="vpad", bufs=4)
                nc.gpsimd.memset(vp, 0.0)
                vpad.append(vp)
            vpad_a[b] = vpad
        ps = pp.tile([128, 512], F32, name="ps_vcl", tag="ps_mm", bufs=1)
        for kc in range(2):
            nc.tensor.matmul(
                ps, wvT[kc][:, ct * 128:(ct + 1) * 128],
                xt_a[b][kc][:, n * 512:(n + 1) * 512],
                start=(kc == 0), stop=(kc == 1))
        nc.vector.tensor_scalar_add(
            out=vpad_a[b][ct][:, 2 + n * 16:2 + (n + 1) * 16, 2:34],
            in0=ps.rearrange("p (h w) -> p h w", h=16),
            scalar1=bv[:, ct:ct + 1])

    lepe_ps = {}

    def emit_lepe_seg(b, ct, half, seg):
        # lepe depthwise conv, 5 taps per filler subunit.
        if ct == 0 and half == 0 and seg == 0:
            lepe_a[b] = [sb.tile([128, L], BF16, name=f"lepe_b{b}c{c2}",
                                 tag="lepe", bufs=4) for c2 in range(2)]
        vp = vpad_a[b][ct]
        if seg == 0:
            lepe_ps[(b, ct, half)] = pp.tile([128, 512], F32, name="ps_lepe",
                                             tag="ps_lp", bufs=1)
        ps = lepe_ps[(b, ct, half)]
        for tap in range(5 * seg, 5 * seg + 5):
            dy, dx = tap // 5, tap % 5
            rhs = vp[:, dy + half * 16:dy + half * 16 + 16, dx:dx + 32]
            nc.tensor.matmul(ps, diag[ct][:, tap, :], rhs,
                             start=(tap == 0), stop=(tap == 24),
                             skip_group_check=True)
        if seg == 4:
            nc.vector.tensor_scalar_add(
                out=lepe_a[b][ct][:, half * 512:(half + 1) * 512], in0=ps,
                scalar1=blepe[:, ct:ct + 1])

    def lepe_units(b):
        return [lambda ct=ct, half=half, seg=seg: emit_lepe_seg(b, ct, half, seg)
                for ct in range(2) for half in range(2) for seg in range(5)]

    def emit_scores_group(b, hg, mt, n, pair):
        # one head-pair of group hg via 2 concurrent row-tiled matmuls
        # (K=32 each) into a 2-bank PSUM tile, evacuated by one merged exp
        # ACT (N=1024).  Adjacent pairs put 4 row-bands in flight.
        if (b, hg) not in es_a:
            es_a[(b, hg)] = {}
        qt = qk_a[b][hg]
        kt = qk_a[b][2 + hg]
        ps2 = pp.tile([128, 2, 512], F32, name="ps2", tag="ps2", bufs=2)
        for i2 in range(2):
            i = pair * 2 + i2
            nc.tensor.matmul(
                ps2[:, i2, :],
                kt[32 * i:32 * i + 32, mt * 128:(mt + 1) * 128],
                qt[32 * i:32 * i + 32, n * 512:(n + 1) * 512],
                start=True, stop=True, tile_position=(32 * i, 0))
        es_t = sb.tile([128, 2, 512], ES_DT,
                       name=f"es_b{b}g{hg}m{mt}n{n}p{pair}",
                       tag="es", bufs=ES_BUFS)
        nc.scalar.activation(out=es_t, in_=ps2, func=AF.Exp, scale=SCALE,
                             bias=esshift)
        es_a[(b, hg)][(mt, n, pair)] = es_t

    # ---------------- incremental PV ----------------
    pv_ps = {}

    def pv_step(b, hg, half, mc):
        # extend the 2 chain tiles (lt pairs) of this half by one mc step:
        # 16 matmuls (4 lt x 4 heads), K=128, N=33, full-array utilization
        es = es_a[(b, hg)]
        if b not in y_a:
            y_a[b] = [sb.tile([128, C], BF16, name=f"y_b{b}l{l2}", tag="y",
                              bufs=16) for l2 in range(8)]
        for lt in range(half * 4, half * 4 + 4):
            pk = (b, hg, lt // 2)
            if mc == 0 and lt % 2 == 0:
                pv_ps[pk] = pp.tile([128, 2, 4, HD + 1], F32, name="ps_pv",
                                    tag="ps_pv", bufs=2)
            ps_pv = pv_ps[pk]
            for i in range(4):
                h = hg * 4 + i
                lhsT = es[(mc, half, i // 2)][
                    :, i % 2, (lt % 4) * 128:(lt % 4) * 128 + 128]
                # 8 chains share this bank and start=True lazily zeroes the
                # WHOLE 2KB region: only the bank's very first matmul says
                # start; every other chain's first write lands on a
                # pending-zero byte and replaces (= its own start).
                nc.tensor.matmul(ps_pv[:, lt % 2, i, :], lhsT,
                                 vT_a[b][mc][:, h, :],
                                 start=(mc == 0 and i == 0 and lt % 2 == 0),
                                 stop=(mc == 7),
                                 skip_group_check=True)

    def pv_evac(b, hg, pi, scalar_norm=False):
        # normalize lt pair (2pi, 2pi+1) straight out of PSUM.  In the final
        # tail the per-head normalize runs as ScalarE ACTs (idle there) to
        # get off the DVE critical chain.
        ps_pv = pv_ps.pop((b, hg, pi))
        for k in (2 * pi, 2 * pi + 1):
            rcp4 = sb.tile([128, 4], F32, name="rcp4", tag="rcp4", bufs=8)
            nc.vector.reciprocal(out=rcp4, in_=ps_pv[:, k % 2, :, HD])
            if scalar_norm:
                for i in range(4):
                    nc.scalar.activation(
                        out=y_a[b][k][:, hg * 128 + i * HD:
                                      hg * 128 + (i + 1) * HD],
                        in_=ps_pv[:, k % 2, i, 0:HD],
                        func=AF.Identity, scale=rcp4[:, i:i + 1])
            else:
                nc.vector.tensor_tensor(
                    out=y_a[b][k][:, hg * 128:(hg + 1) * 128].rearrange(
                        "p (h d) -> p h d", h=4),
                    in0=ps_pv[:, k % 2, :, 0:HD],
                    in1=rcp4.rearrange("p (h o) -> p h o", o=1).broadcast_to(
                        [128, 4, HD]),
                    op=AluOpType.mult)

    # ---------------- LN helpers ----------------
    st1_a, ln1_a, st2_a, ln2_a = {}, {}, {}, {}

    def emit_rsqrt(out_ap, var_ap):
        # out = (var + eps)^-0.5 on VectorE: quake seed + 2 Newton steps.
        g = out_ap.shape[-1]
        vp = sb.tile([128, 8], F32, name="rsq_vp", tag="rsq_vp", bufs=4)
        t = sb.tile([128, 8], F32, name="rsq_t", tag="rsq_t", bufs=4)
        vps = vp[:, 0:g]; ts = t[:, 0:g]
        nc.vector.tensor_scalar_add(out=vps, in0=var_ap, scalar1=epsc)
        nc.vector.tensor_scalar(
            out=ts.bitcast(I32), in0=vps.bitcast(I32), scalar1=1,
            scalar2=None, op0=AluOpType.logical_shift_right)
        nc.vector.tensor_scalar(
            out=out_ap.bitcast(I32), in0=ts.bitcast(I32), scalar1=-1,
            scalar2=0x5f3759df, op0=AluOpType.mult, op1=AluOpType.add)
        nc.vector.tensor_scalar_mul(out=vps, in0=vps, scalar1=0.5)
        for _ in range(2):
            nc.vector.tensor_mul(out=ts, in0=out_ap, in1=out_ap)
            nc.vector.tensor_mul(out=ts, in0=ts, in1=vps)
            nc.vector.tensor_scalar(out=ts, in0=ts, scalar1=-1.0, scalar2=1.5,
                                    op0=AluOpType.mult, op1=AluOpType.add)
            nc.vector.tensor_mul(out=out_ap, in0=out_ap, in1=ts)

    def ln2grp(b, g):
        if b not in ln2_a:
            ln2_a[b] = (
                sb.tile([128, 8, 2], F32, name=f"mv8b_b{b}", tag="mv8", bufs=4),
                sb.tile([128, 8], F32, name=f"rs8b_b{b}", tag="rs8", bufs=4))
        mv8b, rs8b = ln2_a[b]
        gs = slice(g * 4, g * 4 + 4)
        for lt in range(g * 4, g * 4 + 4):
            nc.vector.bn_aggr(out=mv8b[:, lt, :], in_=st2_a[b][:, lt, :])
        emit_rsqrt(rs8b[:, gs], mv8b[:, gs, 1])

    def ln1ap(b, lt, eng=None):
        # LN1 normalize + gate for one l-tile; the 2 elementwise muls/adds
        # can run on GpSimd to split the final-tail chain across engines
        # (bn_stats stays on VectorE).
        y = y_a[b]
        mv8, rs8 = ln1_a[b]
        ee = eng or nc.vector
        if b not in st2_a:
            st2_a[b] = sb.tile([128, 8, 6], F32, name=f"st8b_b{b}", tag="st8",
                               bufs=4)
        gate2_t, bg2_t = gate_a[b][lt]
        nc.vector.tensor_scalar(
            out=y[lt], in0=y[lt], scalar1=mv8[:, lt, 0:1],
            scalar2=rs8[:, lt:lt + 1],
            op0=AluOpType.subtract, op1=AluOpType.mult)
        ee.tensor_mul(out=y[lt], in0=y[lt], in1=gate2_t)
        ee.tensor_add(out=y[lt], in0=y[lt], in1=bg2_t)
        nc.vector.bn_stats(out=st2_a[b][:, lt, :], in_=y[lt])

    def ln1grp(b, g):
        # aggregate LN1 stats for l-tiles [4g, 4g+4)
        if b not in ln1_a:
            ln1_a[b] = (
                sb.tile([128, 8, 2], F32, name=f"mv8_b{b}", tag="mv8", bufs=4),
                sb.tile([128, 8], F32, name=f"rs8_b{b}", tag="rs8", bufs=4))
        mv8, rs8 = ln1_a[b]
        gs = slice(g * 4, g * 4 + 4)
        for l2 in range(g * 4, g * 4 + 4):
            nc.vector.bn_aggr(out=mv8[:, l2, :], in_=st1_a[b][:, l2, :])
        emit_rsqrt(rs8[:, gs], mv8[:, gs, 1])

    def emit_tail_lt(b, lt, final=False):
        # after both head-groups' PV for lt: lepe transpose-add + LN1 stats.
        # Both ct transposes land in one psum tile so a single merged DVE
        # add covers all 256 channels.
        y = y_a[b]
        if b not in st1_a:
            st1_a[b] = sb.tile([128, 8, 6], F32, name=f"st8_b{b}", tag="st8",
                               bufs=4)
        if final:
            ps = pp.tile([128, 2, 128], BF16, name="ps_trf", tag="ps2",
                         bufs=2)
        else:
            ps = pp.tile([128, 2, 128], BF16, name="ps_tr", tag="ps_mm",
                         bufs=1)
        for ct in range(2):
            nc.tensor.transpose(ps[:, ct, :],
                                lepe_a[b][ct][:, lt * 128:(lt + 1) * 128],
                                ident)
        nc.vector.tensor_add(out=y[lt], in0=y[lt],
                             in1=ps.rearrange("p a b -> p (a b)"))
        nc.vector.bn_stats(out=st1_a[b][:, lt, :], in_=y[lt])

    y2T_a = {}

    def lnhalf_lt(b, lt, final=False):
        # LN2 normalize + transpose to [c, l] for one l-tile
        y = y_a[b]
        mv8b, rs8b = ln2_a[b]
        if b not in y2T_a:
            y2T_a[b] = [sb.tile([128, L], BF16, name=f"y2T_b{b}c{ct2}",
                                tag="y2T", bufs=4) for ct2 in range(2)]
        y2T = y2T_a[b]
        y2b = sb.tile([128, C], BF16, name="y2b", tag="y2b", bufs=8)
        nc.vector.tensor_scalar(
            out=y2b, in0=y[lt], scalar1=mv8b[:, lt, 0:1],
            scalar2=rs8b[:, lt:lt + 1],
            op0=AluOpType.subtract, op1=AluOpType.mult)
        for ct in range(2):
            if final:
                ps = pp.tile([128, 128], BF16, name="ps_tr2b",
                             tag="ps2", bufs=2)
            else:
                ps = pp.tile([128, 128], BF16, name="ps_tr2",
                             tag="ps_mm", bufs=1)
            nc.tensor.transpose(
                ps, y2b[:, ct * 128:(ct + 1) * 128], ident)
            dst = y2T[ct][:, lt * 128:(lt + 1) * 128]
            if final:
                nc.scalar.copy(out=dst, in_=ps)
            else:
                nc.vector.tensor_copy(out=dst, in_=ps)

    def emit_proj(b, mt, n, final=False):
        y2T = y2T_a[b]
        o_t = sb.tile([128, 512], F32, name=f"o_b{b}m{mt}n{n}", tag="osb",
                      bufs=2)
        if final:
            ps = pp.tile([128, 512], F32, name="ps_projf", tag="ps_pv", bufs=2)
        else:
            ps = pp.tile([128, 512], F32, name="ps_proj", tag="ps_mm", bufs=1)
        for kc in range(2):
            nc.tensor.matmul(
                ps, wprojT[kc][:, mt * 128:(mt + 1) * 128],
                y2T[kc][:, n * 512:(n + 1) * 512],
                start=(kc == 0), stop=(kc == 1))
        nc.vector.tensor_scalar_add(
            out=o_t, in0=ps, scalar1=bproj[:, mt:mt + 1])
        oeng = (nc.sync, nc.gpsimd, nc.scalar)[(b * 2 + mt + n) % 3]
        oeng.dma_start(
            out=io['out'][b, mt * 128:(mt + 1) * 128,
                          n * 512:(n + 1) * 512],
            in_=o_t)

    # ---------------- schedule ----------------
    def sc_phase(b, hg, fillers, pinned=None, pre=None, fstart=0):
        # 32 head-pair scores groups in (n, mt) order; each mt's 2 groups are
        # emitted adjacently (4 row-bands in flight).  The incremental PV
        # chains for half n step at mt+1 (one group behind the exp stream);
        # half 0's last step + evac overlap the start of half 1.  `pinned`
        # maps jp -> [units] run right after that jp's groups; `fillers` are
        # spread evenly over jps [fstart, 16).
        fi = 0
        pinned = pinned or {}
        if pre:
            for u in pre:
                u()
        for n in (0, 1):
            for mt in range(8):
                jp = n * 8 + mt
                for pr in (0, 1):
                    emit_scores_group(b, hg, mt, n, pr)
                if jp == 8:
                    pv_step(b, hg, 0, 7)
                    pv_evac(b, hg, 0)
                    pv_evac(b, hg, 1)
                if mt >= 1:
                    pv_step(b, hg, n, mt - 1)
                for u in pinned.get(jp, ()):
                    u()
                want = max(0, jp + 1 - fstart) * len(fillers) // (16 - fstart)
                while fi < want:
                    fillers[fi](); fi += 1
        while fi < len(fillers):
            fillers[fi](); fi += 1

    def phase_end(b, hg, scalar_norm=False):
        # close half 1: final mc step + evacuation of lt pairs 2,3.
        # Runs pinned at jp0 of the NEXT phase so its es-ACT wait hides
        # behind that phase's first score matmuls.
        pv_step(b, hg, 1, 7)
        pv_evac(b, hg, 2, scalar_norm=scalar_norm)
        pv_evac(b, hg, 3, scalar_norm=scalar_norm)

    def lepe_units_half(b, half):
        return [lambda ct=ct, seg=seg: emit_lepe_seg(b, ct, half, seg)
                for ct in range(2) for seg in range(5)]

    # ---- batch 0, head-group 0 ----
    # minimal pre-critical-path: hg0 q/k tiles (m=0 q, m=2 k), rope, vg(0,0);
    # these borrow the idle ps2 ring (big_ps) to avoid ps_mm serialization
    emit_qk_conv(0, 0, scalar_evac=True, big_ps=True)
    emit_qk_conv(0, 2, scalar_evac=True, big_ps=True)
    emit_rope(0, 0, dve_mul=True, big_ps=True, halves=(0,))
    emit_rope(0, 2, dve_mul=True, big_ps=True, halves=(0,))
    emit_rope(0, 0, dve_mul=True, big_ps=True, halves=(1,))
    emit_rope(0, 2, dve_mul=True, big_ps=True, halves=(1,))

    F0 = [lambda: emit_qk_conv(0, 1), lambda: emit_qk_conv(0, 3),
          lambda: emit_rope(0, 1), lambda: emit_rope(0, 3)]
    for ct in range(2):
        for n in range(2):
            F0.append(lambda ct=ct, n=n: emit_vcl(0, ct, n))
    F0.extend(lepe_units_half(0, 0))
    # vg(0, mt) pinned at jp=mt so chain step mc=mt (at jp=mt+1) has its vT;
    # vg(0,0) sits AFTER jp0's groups so its rhsvg-DMA wait can't stall the
    # in-order PE queue ahead of the first scores
    pin0 = {mt: [lambda mt=mt: emit_vg(0, mt)] for mt in range(8)}
    sc_phase(0, 0, F0, pinned=pin0)

    # ---- batch 0, head-group 1 ----
    # fillers: b1 q/k prep, b1 v image, b0 lepe second half; pinned: b1 vg
    # early, then b0's lt0-3 tail pipeline in per-lt/per-group units
    F0b = []
    for m in (0, 2, 1, 3):
        F0b.append(lambda m=m: emit_qk_conv(1, m))
    for t in (0, 2, 1, 3):
        F0b.append(lambda t=t: emit_rope(1, t))
    for ct in range(2):
        for n in range(2):
            F0b.append(lambda ct=ct, n=n: emit_vcl(1, ct, n))
    F0b.extend(lepe_units_half(0, 1))
    pin1 = {0: [lambda: phase_end(0, 0)]}
    for mt in range(8):
        pin1.setdefault(mt + 1, []).append(lambda mt=mt: emit_vg(1, mt))
    for i, lt in enumerate(range(4)):
        pin1.setdefault(9 + i, []).append(lambda lt=lt: emit_tail_lt(0, lt))
    pin1.setdefault(13, []).append(lambda: ln1grp(0, 0))
    pin1.setdefault(14, []).extend([lambda: ln1ap(0, 0),
                                    lambda: ln1ap(0, 1, eng=nc.gpsimd)])
    pin1.setdefault(15, []).extend([lambda: ln1ap(0, 2),
                                    lambda: ln1ap(0, 3, eng=nc.gpsimd)])
    sc_phase(0, 1, F0b, pinned=pin1)

    # ---- batch 1, head-group 0 ----
    # pinned: b0's remaining endgame spread one small unit per jp; fillers:
    # b1's lepe (its ps_lp bank no longer shared with tail transposes)
    pin2 = {0: [lambda: phase_end(0, 1), lambda: ln2grp(0, 0)],
            1: [lambda: lnhalf_lt(0, 0), lambda: lnhalf_lt(0, 1)],
            2: [lambda: lnhalf_lt(0, 2), lambda: lnhalf_lt(0, 3)],
            3: [lambda: emit_proj(0, 0, 0), lambda: emit_proj(0, 1, 0)],
            4: [lambda: emit_tail_lt(0, 4)],
            5: [lambda: emit_tail_lt(0, 5)],
            6: [lambda: emit_tail_lt(0, 6)],
            7: [lambda: emit_tail_lt(0, 7)],
            8: [lambda: ln1grp(0, 1)],
            9: [lambda: ln1ap(0, 4),
                lambda: ln1ap(0, 5, eng=nc.gpsimd)],
            10: [lambda: ln1ap(0, 6),
                 lambda: ln1ap(0, 7, eng=nc.gpsimd)],
            11: [lambda: ln2grp(0, 1)],
            12: [lambda: lnhalf_lt(0, 4), lambda: lnhalf_lt(0, 5)],
            13: [lambda: lnhalf_lt(0, 6), lambda: lnhalf_lt(0, 7)],
            14: [lambda: emit_proj(0, 0, 1), lambda: emit_proj(0, 1, 1)]}
    sc_phase(1, 0, lepe_units_half(1, 0), pinned=pin2)

    # ---- batch 1, head-group 1 ----
    pin3 = {0: [lambda: phase_end(1, 0)]}
    for i, lt in enumerate(range(4)):
        pin3.setdefault(9 + i, []).append(lambda lt=lt: emit_tail_lt(1, lt))
    pin3.setdefault(13, []).append(lambda: ln1grp(1, 0))
    pin3.setdefault(14, []).extend([lambda: ln1ap(1, 0),
                                    lambda: ln1ap(1, 1, eng=nc.gpsimd)])
    pin3.setdefault(15, []).extend([lambda: ln1ap(1, 2),
                                    lambda: ln1ap(1, 3, eng=nc.gpsimd)])
    sc_phase(1, 1, lepe_units_half(1, 1), pinned=pin3)

    # ---- final tail: interleave b1-half0's LN2/proj (independent) with
    # b1-half1's tails so PE, DVE, GpSimd and ScalarE all overlap
    phase_end(1, 1, scalar_norm=True)
    ln2grp(1, 0)
    emit_tail_lt(1, 4, final=True)
    emit_tail_lt(1, 5, final=True)
    emit_tail_lt(1, 6, final=True)
    emit_tail_lt(1, 7, final=True)
    lnhalf_lt(1, 0, final=True)
    lnhalf_lt(1, 1, final=True)
    lnhalf_lt(1, 2, final=True)
    lnhalf_lt(1, 3, final=True)
    emit_proj(1, 0, 0, final=True)
    emit_proj(1, 1, 0, final=True)
    ln1grp(1, 1)
    ln1ap(1, 4)
    ln1ap(1, 5, eng=nc.gpsimd)
    ln1ap(1, 6)
    ln1ap(1, 7, eng=nc.gpsimd)
    ln2grp(1, 1)
    for lt in range(4, 8):
        lnhalf_lt(1, lt, final=True)
    # last projections in column-quarters: matmul/evac/DMA of consecutive
    # chunks pipeline instead of serializing the drain
    for q in range(2):
        for mt in range(2):
            o_t = sb.tile([128, 256], F32, name=f"o_fq{mt}{q}", tag="osb",
                          bufs=2)
            ps = pp.tile([128, 256], F32, name="ps_projq", tag="ps_pv",
                         bufs=2)
            for kc in range(2):
                nc.tensor.matmul(
                    ps, wprojT[kc][:, mt * 128:(mt + 1) * 128],
                    y2T_a[1][kc][:, 512 + q * 256:512 + (q + 1) * 256],
                    start=(kc == 0), stop=(kc == 1))
            nc.scalar.activation(out=o_t, in_=ps, func=AF.Identity,
                                 bias=bproj[:, mt:mt + 1], scale=1.0)
            oeng = (nc.sync, nc.gpsimd, nc.scalar)[(mt + q) % 3]
            oeng.dma_start(
                out=io['out'][1, mt * 128:(mt + 1) * 128,
                              512 + q * 256:512 + (q + 1) * 256],
                in_=o_t)


# ----------------------------------------------------------------------
# host side
# ----------------------------------------------------------------------
def host_prep(inp):
    f32 = np.float32
    bf = lambda a: np.ascontiguousarray(a).astype(NPBF)
    p = {}
    w_qkv = np.asarray(inp['w_qkv'], f32)
    b_qkv = np.asarray(inp['b_qkv'], f32)
    # q/k weights with 4-heads-per-tile packing: head h -> tile h//4,
    # partition offset 32*(h%4); k block starts at column 256.
    wqk_pad = np.zeros((C, 512), f32)
    bqk_pad = np.zeros(512, f32)
    for h in range(NH):
        dst = (h // 4) * 128 + (h % 4) * 32
        wqk_pad[:, dst:dst + 32] = w_qkv[h * 32:(h + 1) * 32].T
        wqk_pad[:, 256 + dst:256 + dst + 32] = \
            w_qkv[256 + h * 32:256 + (h + 1) * 32].T
        bqk_pad[dst:dst + 32] = b_qkv[h * 32:(h + 1) * 32]
        bqk_pad[256 + dst:256 + dst + 32] = b_qkv[256 + h * 32:256 + (h + 1) * 32]
    p['wqkT'] = bf(wqk_pad)
    p['bqk'] = np.ascontiguousarray(bqk_pad.reshape(4, 128).T)
    p['wvT'] = bf(w_qkv[512:].T)
    p['bv'] = np.ascontiguousarray(b_qkv[512:].reshape(2, 128).T)
    s = np.asarray(inp['bn_gamma'], f32) / np.sqrt(np.float32(1.0) + f32(BN_EPS))
    wg = np.asarray(inp['w_gate'], f32) * s[:, None]
    bg = np.asarray(inp['b_gate'], f32) * s + np.asarray(inp['bn_beta'], f32)
    p['rhsvg'] = bf(np.concatenate([w_qkv[512:].T, wg.T], axis=1))
    p['bvgbc'] = bf(np.tile(np.concatenate([b_qkv[512:], bg])[None, :], (128, 1)))
    wp = np.asarray(inp['w_proj'], f32) * np.asarray(inp['ln_gamma'], f32)[None, :]
    bp = (np.asarray(inp['b_proj'], f32)
          + np.asarray(inp['w_proj'], f32) @ np.asarray(inp['ln_beta'], f32))
    p['wprojT'] = bf(wp.T)
    p['bproj'] = np.ascontiguousarray(bp.reshape(2, 128).T)
    cosl = np.asarray(inp['cos'], f32).reshape(L, HD).T
    sinl = np.asarray(inp['sin'], f32).reshape(L, HD).T
    p['cosq'] = bf(np.tile(cosl, (4, 1)))
    p['sinq'] = bf(np.tile(sinl, (4, 1)))
    R = np.zeros((128, 128), f32)
    for i in range(64):
        R[2 * i + 1, 2 * i] = -1.0
        R[2 * i, 2 * i + 1] = 1.0
    p['rotmat'] = bf(R)
    p['ident'] = bf(np.eye(128, dtype=f32))
    # diag[ct, :, tap*128:(tap+1)*128] = diag(w5[ct, :, tap])
    w5 = np.asarray(inp['w_lepe'], f32).reshape(2, 128, 25)
    dw = np.zeros((2, 128, 25 * 128), f32)
    idx = np.arange(128)
    for ct in range(2):
        for tap in range(25):
            dw[ct, idx, tap * 128 + idx] = w5[ct, :, tap]
    p['diagw'] = bf(dw)
    p['blepe'] = np.ascontiguousarray(
        np.asarray(inp['b_lepe'], f32).reshape(2, 128).T)
    # gate is computed as g*(1+tanh(g/2)) = 2*silu(g); the 0.5 is folded here
    p['g1bc'] = np.tile(0.5 * np.asarray(inp['norm_gamma'], f32)[None, :], (128, 1))
    p['b1bc'] = np.tile(0.5 * np.asarray(inp['norm_beta'], f32)[None, :], (128, 1))
    return p


_NC = None


def _get_nc():
    global _NC
    if _NC is None:
        _NC = build_program()
    return _NC


def make_in_maps(inputs):
    p = host_prep(inputs)
    x = np.asarray(inputs['x'], np.float32).reshape(B, C, L)
    in_maps = []
    for i in range(NCORES):
        m = dict(p)
        m['x2'] = np.ascontiguousarray(x[i * BPC:(i + 1) * BPC]).astype(NPBF)
        in_maps.append(m)
    return in_maps


def kernel(**inputs):
    from concourse.bass_utils import run_bass_kernel_spmd
    nc = _get_nc()
    in_maps = make_in_maps(inputs)
    res = run_bass_kernel_spmd(nc, in_maps, core_ids=list(range(NCORES)))
    outs = [np.asarray(res.results[i]['out'], np.float32).reshape(BPC, C, H, W)
            for i in range(NCORES)]
    return np.concatenate(outs, axis=0)


# revision 32
# speedup vs baseline: 1.0047x; 1.0047x over previous
"""Trainium2 Bass kernel for nn_Attention_25572235280790.

Dense attention block (B=16, C=256, H=W=32, NH=8, HD=32) with RoPE-style theta
shift, LePE depthwise 5x5 conv, BN+SiLU gate, channel LayerNorms and 1x1 convs.

Sharding: data-parallel over batch across 8 NeuronCores (2 batches/core), no
collectives.  Inside each core everything is computed in two layouts:
  - [c, l]  (channels on partitions)  for the 1x1 convs / scores / lepe
  - [l, c]  (spatial on partitions)   for softmax-normalize / layernorms / gate

v4 schedule (255.7us -> 228.5us): PV runs as INCREMENTAL accumulation chains
inside each head-group's own scores phase, lagging one mt-group behind the
exp stream, so no PV lump ever trails the scores.  Chains share PSUM banks
without per-chain start=True (a start lazily zeroes the whole 2KB region,
wiping sibling chains' first contribution): only each bank's first matmul
starts; every other chain's first write replaces via the pending-zero flags.
Each l-tile half's LN1/LN2/proj tail is split into per-lt units pinned one
per scores-jp in the NEXT phase; batch-0's endgame runs as fillers inside
batch-1's first phase.  The trailing half's tails split into column halves:
the left half (hg0 attn + lepe ct0) runs early in the last phase with
partial bn_stats, aggregated against the late right half via 2-block
bn_aggr.  Startup DMAs are chunked so the first conv's exact inputs land
first, striped over the sync/gpsimd/scalar queues.  Scores S^T[m, l] for 4
heads are concurrent row-tiled matmuls (tile_position=(32i,0), K=32, bus
limit 2 col-transfers/cycle) into 2-bank PSUM pairs evacuated by merged exp
ACTs (N=1024) on ScalarE.  LN rsqrt is quake-seed + 1 Newton step on
VectorE so ScalarE never leaves the exp/tanh table set; the final tail's
PSUM evacuations and copies run on ScalarE (idle there), never GpSimd
(no PSUM access, ~4x slower tensor ops).
"""

import numpy as np
import ml_dtypes

import concourse.bass as bass
import concourse.tile as tile
from concourse import bacc
from concourse import mybir
from concourse.alu_op_type import AluOpType

B, C, H, W = 16, 256, 32, 32
NH, HD = 8, 32
SCALE = HD ** -0.5
LN_EPS = 1e-6
BN_EPS = 1e-5
L = H * W
NCORES = 8
BPC = B // NCORES          # batches per core
AF = mybir.ActivationFunctionType
F32 = mybir.dt.float32
BF16 = mybir.dt.bfloat16
I32 = mybir.dt.int32

NPBF = ml_dtypes.bfloat16
# es tiles are fp8-e3m4: softmax is shift-invariant, so exp(s*SCALE - 0.5)
# keeps the observed range [0.057, 20.9] inside e3m4's [~0.008, 15.5] while
# e3m4's 4 mantissa bits halve the quantization error of e4m3.
ES_DT = mybir.dt.float8e3
ES_SHIFT = -0.5
ES_BUFS = 12


def build_program():
    nc = bacc.Bacc()
    dp = nc.declare_dram_parameter
    io = {
        'x2':      dp('x2',      [BPC, C, L],   BF16, isOutput=False),
        'wqkT':    dp('wqkT',    [C, 512],      BF16, isOutput=False),
        'bqk':     dp('bqk',     [128, 4],      F32,  isOutput=False),
        'wvT':     dp('wvT',     [C, C],        BF16, isOutput=False),
        'bv':      dp('bv',      [128, 2],      F32,  isOutput=False),
        'rhsvg':   dp('rhsvg',   [C, 512],      BF16, isOutput=False),
        'bvgbc':   dp('bvgbc',   [128, 512],    BF16, isOutput=False),
        'wprojT':  dp('wprojT',  [C, C],        BF16, isOutput=False),
        'bproj':   dp('bproj',   [128, 2],      F32,  isOutput=False),
        'cosq':    dp('cosq',    [128, L],      BF16, isOutput=False),
        'sinq':    dp('sinq',    [128, L],      BF16, isOutput=False),
        'rotmat':  dp('rotmat',  [128, 128],    BF16, isOutput=False),
        'ident':   dp('ident',   [128, 128],    BF16, isOutput=False),
        'diagw':   dp('diagw',   [2, 128, 25 * 128], BF16, isOutput=False),
        'blepe':   dp('blepe',   [128, 2],      F32,  isOutput=False),
        'g1bc':    dp('g1bc',    [128, C],      F32,  isOutput=False),
        'b1bc':    dp('b1bc',    [128, C],      F32,  isOutput=False),
        'out':     dp('out',     [BPC, C, L],   F32,  isOutput=True),
    }
    with tile.TileContext(nc) as tc:
        _emit(tc, io)
    nc.compile()
    return nc


def _emit(tc, io):
    with (tc.tile_pool(name="cw", bufs=1) as cw,
          tc.tile_pool(name="sb", bufs=2) as sb,
          tc.tile_pool(name="pp", bufs=2, space="PSUM") as pp):
        _emit_body(tc, io, cw, sb, pp)


def _emit_body(tc, io, cw, sb, pp):
    nc = tc.nc

    # ------------------------------------------------------------------
    # persistent constants -- startup DMAs fan out across engine queues
    # (each engine's sequencer owns its own DMA ring; Pool issue is cheap)
    # ------------------------------------------------------------------
    dma_engs = [nc.sync, nc.gpsimd, nc.scalar]
    _dma_rr = [0]

    def dma(out, in_, eng=None):
        e = dma_engs[_dma_rr[0] % len(dma_engs)] if eng is None else eng
        _dma_rr[0] += 1
        e.dma_start(out=out, in_=in_)

    def cload(name, dtype, eng=None):
        src = io[name]
        t = cw.tile(list(src.shape), dtype, name=f"c_{name}")
        dma(out=t, in_=src[:], eng=eng)
        return t

    def cload2(name, dtype, cols, engs=(None, None)):
        ts = [cw.tile([128, cols], dtype, name=f"c_{name}{i}") for i in range(2)]
        for i in range(2):
            dma(out=ts[i], in_=io[name][i * 128:(i + 1) * 128, :], eng=engs[i])
        return ts

    # first wave, chunked so the first qk conv's exact inputs (wqkT m-cols
    # 0:128 + 256:384, x l-cols 0:512) land first, striped over all 3 queues
    wqkT = [cw.tile([128, 512], BF16, name=f"c_wqkT{i}") for i in range(2)]
    xt_a = {0: [], 1: []}
    for b_ in (0,):
        for ct_ in range(2):
            xt_a[b_].append(sb.tile([128, L], BF16, name=f"x_b{b_}c{ct_}",
                                    tag="xt", bufs=4))
    q3 = (nc.sync, nc.gpsimd, nc.scalar)
    first_wave = [
        (wqkT[0][:, 0:128],   io['wqkT'][0:128, 0:128]),
        (wqkT[1][:, 0:128],   io['wqkT'][128:256, 0:128]),
        (wqkT[0][:, 256:384], io['wqkT'][0:128, 256:384]),
        (wqkT[1][:, 256:384], io['wqkT'][128:256, 256:384]),
        (xt_a[0][0][:, 0:512], io['x2'][0, 0:128, 0:512]),
        (xt_a[0][1][:, 0:512], io['x2'][0, 128:256, 0:512]),
        (xt_a[0][0][:, 512:L], io['x2'][0, 0:128, 512:L]),
        (xt_a[0][1][:, 512:L], io['x2'][0, 128:256, 512:L]),
        (wqkT[0][:, 128:256], io['wqkT'][0:128, 128:256]),
        (wqkT[1][:, 128:256], io['wqkT'][128:256, 128:256]),
        (wqkT[0][:, 384:512], io['wqkT'][0:128, 384:512]),
        (wqkT[1][:, 384:512], io['wqkT'][128:256, 384:512]),
    ]
    for j, (dst, src) in enumerate(first_wave[:8]):
        dma(out=dst, in_=src, eng=q3[j % 3])

    def load_x(b_, engs):
        for ct_ in range(2):
            x_t = sb.tile([128, L], BF16, name=f"x_b{b_}c{ct_}", tag="xt",
                          bufs=4)
            dma(out=x_t, in_=io['x2'][b_, ct_ * 128:(ct_ + 1) * 128, :],
                eng=engs[ct_])
            xt_a[b_].append(x_t)
    bqk = cload('bqk', F32, eng=nc.gpsimd)
    rotmat = cload('rotmat', BF16, eng=nc.scalar)
    # rope's first-half tables + rhsvg (gates vg at jp0) land before the
    # deferred wqkT b-chunks (qk fillers at jp1-2) and everything else
    cosq = cw.tile([128, L], BF16, name="c_cosq")
    sinq = cw.tile([128, L], BF16, name="c_sinq")
    dma(out=cosq[:, 0:512], in_=io['cosq'][:, 0:512], eng=nc.gpsimd)
    dma(out=sinq[:, 0:512], in_=io['sinq'][:, 0:512], eng=nc.sync)
    rhsvg = cload2('rhsvg', BF16, 512, engs=(nc.scalar, nc.sync))
    for j, (dst, src) in enumerate(first_wave[8:]):
        dma(out=dst, in_=src, eng=q3[(j + 1) % 3])
    dma(out=cosq[:, 512:L], in_=io['cosq'][:, 512:L], eng=nc.scalar)
    dma(out=sinq[:, 512:L], in_=io['sinq'][:, 512:L], eng=nc.gpsimd)
    bvgbc = cload('bvgbc', BF16, eng=nc.gpsimd)
    wvT = cload2('wvT', BF16, C, engs=(nc.scalar, nc.gpsimd))
    bv = cload('bv', F32)
    ident = cload('ident', BF16)
    blepe = cload('blepe', F32)
    g1bc = cload('g1bc', F32)
    b1bc = cload('b1bc', F32)
    wprojT = cload2('wprojT', BF16, C)
    bproj = cload('bproj', F32)
    load_x(1, (nc.sync, nc.gpsimd))

    epsc = cw.tile([128, 1], F32, name="epsc")
    nc.gpsimd.memset(epsc, LN_EPS)
    esshift = cw.tile([128, 1], F32, name="esshift")
    nc.gpsimd.memset(esshift, ES_SHIFT)
    # dummy exp as the first ScalarE instruction: pulls the ~1.3us ACT
    # table load into the DMA window instead of the first evacuation
    warmup = cw.tile([128, 1], F32, name="warmup")
    nc.scalar.activation(out=warmup, in_=epsc, func=AF.Exp)

    # lepe diagonal stationaries diag[ct][:, tap, :] = diag(w5[ct][:, tap])
    diag = [cw.tile([128, 25, 128], BF16, name=f"diag{i}") for i in range(2)]
    for i in range(2):
        dma(out=diag[i].rearrange("p a b -> p (a b)"), in_=io['diagw'][i],
            eng=(nc.sync, nc.gpsimd)[i])

    qk_a, vT_a, gate_a, vpad_a, lepe_a, y_a = ({} for _ in range(6))
    es_a = {}

    def emit_qk_conv(b, m, scalar_evac=False, big_ps=False):
        # one M-tile of the q/k 1x1 conv; 4 M-tiles: q0 q1 k0 k1.
        # big_ps: pre-phase calls borrow the (idle) ps2 ring so the
        # single-buffered ps_mm tag doesn't serialize the startup chain.
        if b not in qk_a:
            qk_a[b] = {}
        qk_t = sb.tile([128, L], BF16, name=f"qk_b{b}m{m}", tag="qk", bufs=8)
        for n in range(2):
            if big_ps:
                ps = pp.tile([128, 512], F32, name="ps_mmb", tag="ps2", bufs=2)
            else:
                ps = pp.tile([128, 512], F32, name="ps_mm", tag="ps_mm", bufs=1)
            for kc in range(2):
                nc.tensor.matmul(
                    ps, wqkT[kc][:, m * 128:(m + 1) * 128],
                    xt_a[b][kc][:, n * 512:(n + 1) * 512],
                    start=(kc == 0), stop=(kc == 1))
            if scalar_evac:
                nc.scalar.activation(
                    out=qk_t[:, n * 512:(n + 1) * 512], in_=ps,
                    func=AF.Identity, bias=bqk[:, m:m + 1], scale=1.0)
            else:
                nc.vector.tensor_scalar_add(
                    out=qk_t[:, n * 512:(n + 1) * 512], in0=ps,
                    scalar1=bqk[:, m:m + 1])
        qk_a[b][m] = qk_t

    def emit_rope(b, t, dve_mul=False, big_ps=False, halves=(0, 1)):
        # theta shift on q/k tile t, in [d, l] layout.
        for n in halves:
            sl = slice(n * 512, (n + 1) * 512)
            if big_ps:
                ps = pp.tile([128, 512], F32, name="ps_rotb", tag="ps2", bufs=2)
            else:
                ps = pp.tile([128, 512], F32, name="ps_rot", tag="ps_mm", bufs=1)
            nc.tensor.matmul(ps, rotmat, qk_a[b][t][:, sl],
                             start=True, stop=True)
            gtmp = sb.tile([128, 512], BF16, name="rope_g", tag="rope_g", bufs=2)
            mul_eng = nc.vector if dve_mul else nc.gpsimd
            mul_eng.tensor_mul(out=gtmp, in0=qk_a[b][t][:, sl],
                               in1=cosq[:, sl])
            vtmp = sb.tile([128, 512], BF16, name="rope_v", tag="rope_v", bufs=2)
            nc.vector.tensor_mul(out=vtmp, in0=ps, in1=sinq[:, sl])
            nc.vector.tensor_add(out=qk_a[b][t][:, sl], in0=gtmp, in1=vtmp)

    def emit_vg(b, lt):
        # x-stationary pass: v^T and gate^T in [l, .] layout.
        # gate = g*(1+tanh(g/2)) = 2*silu(g); tanh shares the Exp ACT table.
        if lt == 0:
            vT_a[b] = []
            gate_a[b] = []
        ps = pp.tile([128, 512], F32, name="ps_vg", tag="ps_mm", bufs=1)
        for kc in range(2):
            nc.tensor.matmul(
                ps, xt_a[b][kc][:, lt * 128:(lt + 1) * 128], rhsvg[kc],
                start=(kc == 0), stop=(kc == 1))
        vT_t = sb.tile([128, NH, HD + 1], BF16, name=f"vT_b{b}l{lt}",
                       tag="vT", bufs=16)
        nc.gpsimd.memset(vT_t[:, :, HD:HD + 1], 1.0)
        nc.vector.tensor_tensor(
            out=vT_t[:, :, 0:HD],
            in0=ps[:, 0:256].rearrange("p (h d) -> p h d", h=NH),
            in1=bvgbc[:, 0:256].rearrange("p (h d) -> p h d", h=NH),
            op=AluOpType.add)
        vT_a[b].append(vT_t)
        gate_t = sb.tile([128, C], BF16, name=f"gate_b{b}l{lt}",
                         tag="gate_t", bufs=3)
        gb = sb.tile([128, C], F32, name="gb", tag="gb", bufs=4)
        nc.vector.tensor_add(out=gb, in0=ps[:, 256:512], in1=bvgbc[:, 256:512])
        tnt = sb.tile([128, C], F32, name="tnt", tag="tnt", bufs=3)
        nc.scalar.activation(out=tnt, in_=gb, func=AF.Tanh, scale=0.5)
        wt_ = sb.tile([128, C], F32, name="wt_", tag="wt_", bufs=2)
        nc.gpsimd.tensor_mul(out=wt_, in0=gb, in1=tnt)
        nc.gpsimd.tensor_add(out=gate_t, in0=wt_, in1=gb)
        # fold the LN1 affine into the gate here (off the tail critical
        # path): y_ln1 = ((y-mu)*rs)*gate2 + bg2
        gate2_t = sb.tile([128, C], BF16, name=f"gate2_b{b}l{lt}",
                          tag="gate", bufs=16)
        nc.gpsimd.tensor_mul(out=gate2_t, in0=gate_t, in1=g1bc)
        bg2_t = sb.tile([128, C], BF16, name=f"bg2_b{b}l{lt}",
                        tag="bg2", bufs=16)
        nc.gpsimd.tensor_mul(out=bg2_t, in0=gate_t, in1=b1bc)
        gate_a[b].append((gate2_t, bg2_t))

    def emit_vcl(b, ct, n):
        # v in [c, l] (for lepe), into zero-padded image tiles
        if ct == 0 and n == 0:
            vpad = []
            for c2 in range(2):
                vp = sb.tile([128, 36, 36], BF16, name=f"vpad_b{b}c{c2}",
                             tag="vpad", bufs=4)
                nc.gpsimd.memset(vp, 0.0)
                vpad.append(vp)
            vpad_a[b] = vpad
        ps = pp.tile([128, 512], F32, name="ps_vcl", tag="ps_mm", bufs=1)
        for kc in range(2):
            nc.tensor.matmul(
                ps, wvT[kc][:, ct * 128:(ct + 1) * 128],
                xt_a[b][kc][:, n * 512:(n + 1) * 512],
                start=(kc == 0), stop=(kc == 1))
        nc.vector.tensor_scalar_add(
            out=vpad_a[b][ct][:, 2 + n * 16:2 + (n + 1) * 16, 2:34],
            in0=ps.rearrange("p (h w) -> p h w", h=16),
            scalar1=bv[:, ct:ct + 1])

    lepe_ps = {}

    def emit_lepe_seg(b, ct, half, seg):
        # lepe depthwise conv, 5 taps per filler subunit.
        if ct == 0 and half == 0 and seg == 0:
            lepe_a[b] = [sb.tile([128, L], BF16, name=f"lepe_b{b}c{c2}",
                                 tag="lepe", bufs=4) for c2 in range(2)]
        vp = vpad_a[b][ct]
        if seg == 0:
            lepe_ps[(b, ct, half)] = pp.tile([128, 512], F32, name="ps_lepe",
                                             tag="ps_lp", bufs=1)
        ps = lepe_ps[(b, ct, half)]
        for tap in range(5 * seg, 5 * seg + 5):
            dy, dx = tap // 5, tap % 5
            rhs = vp[:, dy + half * 16:dy + half * 16 + 16, dx:dx + 32]
            nc.tensor.matmul(ps, diag[ct][:, tap, :], rhs,
                             start=(tap == 0), stop=(tap == 24),
                             skip_group_check=True)
        if seg == 4:
            nc.vector.tensor_scalar_add(
                out=lepe_a[b][ct][:, half * 512:(half + 1) * 512], in0=ps,
                scalar1=blepe[:, ct:ct + 1])

    def lepe_units(b):
        return [lambda ct=ct, half=half, seg=seg: emit_lepe_seg(b, ct, half, seg)
                for ct in range(2) for half in range(2) for seg in range(5)]

    def emit_scores_group(b, hg, mt, n, pair):
        # one head-pair of group hg via 2 concurrent row-tiled matmuls
        # (K=32 each) into a 2-bank PSUM tile, evacuated by one merged exp
        # ACT (N=1024).  Adjacent pairs put 4 row-bands in flight.
        if (b, hg) not in es_a:
            es_a[(b, hg)] = {}
        qt = qk_a[b][hg]
        kt = qk_a[b][2 + hg]
        ps2 = pp.tile([128, 2, 512], F32, name="ps2", tag="ps2", bufs=2)
        for i2 in range(2):
            i = pair * 2 + i2
            nc.tensor.matmul(
                ps2[:, i2, :],
                kt[32 * i:32 * i + 32, mt * 128:(mt + 1) * 128],
                qt[32 * i:32 * i + 32, n * 512:(n + 1) * 512],
                start=True, stop=True, tile_position=(32 * i, 0))
        es_t = sb.tile([128, 2, 512], ES_DT,
                       name=f"es_b{b}g{hg}m{mt}n{n}p{pair}",
                       tag="es", bufs=ES_BUFS)
        nc.scalar.activation(out=es_t, in_=ps2, func=AF.Exp, scale=SCALE,
                             bias=esshift)
        es_a[(b, hg)][(mt, n, pair)] = es_t

    # ---------------- incremental PV ----------------
    pv_ps = {}

    def pv_step(b, hg, half, mc):
        # extend the 2 chain tiles (lt pairs) of this half by one mc step:
        # 16 matmuls (4 lt x 4 heads), K=128, N=33, full-array utilization
        es = es_a[(b, hg)]
        if b not in y_a:
            y_a[b] = [sb.tile([128, C], BF16, name=f"y_b{b}l{l2}", tag="y",
                              bufs=16) for l2 in range(8)]
        for lt in range(half * 4, half * 4 + 4):
            pk = (b, hg, lt // 2)
            if mc == 0 and lt % 2 == 0:
                pv_ps[pk] = pp.tile([128, 2, 4, HD + 1], F32, name="ps_pv",
                                    tag="ps_pv", bufs=2)
            ps_pv = pv_ps[pk]
            for i in range(4):
                h = hg * 4 + i
                lhsT = es[(mc, half, i // 2)][
                    :, i % 2, (lt % 4) * 128:(lt % 4) * 128 + 128]
                # 8 chains share this bank and start=True lazily zeroes the
                # WHOLE 2KB region: only the bank's very first matmul says
                # start; every other chain's first write lands on a
                # pending-zero byte and replaces (= its own start).
                nc.tensor.matmul(ps_pv[:, lt % 2, i, :], lhsT,
                                 vT_a[b][mc][:, h, :],
                                 start=(mc == 0 and i == 0 and lt % 2 == 0),
                                 stop=(mc == 7),
                                 skip_group_check=True)

    def pv_evac(b, hg, pi, scalar_norm=False):
        # normalize lt pair (2pi, 2pi+1) straight out of PSUM.  In the final
        # tail the per-head normalize runs as ScalarE ACTs (idle there) to
        # get off the DVE critical chain.
        ps_pv = pv_ps.pop((b, hg, pi))
        for k in (2 * pi, 2 * pi + 1):
            rcp4 = sb.tile([128, 4], F32, name="rcp4", tag="rcp4", bufs=8)
            nc.vector.reciprocal(out=rcp4, in_=ps_pv[:, k % 2, :, HD])
            if scalar_norm:
                for i in range(4):
                    nc.scalar.activation(
                        out=y_a[b][k][:, hg * 128 + i * HD:
                                      hg * 128 + (i + 1) * HD],
                        in_=ps_pv[:, k % 2, i, 0:HD],
                        func=AF.Identity, scale=rcp4[:, i:i + 1])
            else:
                nc.vector.tensor_tensor(
                    out=y_a[b][k][:, hg * 128:(hg + 1) * 128].rearrange(
                        "p (h d) -> p h d", h=4),
                    in0=ps_pv[:, k % 2, :, 0:HD],
                    in1=rcp4.rearrange("p (h o) -> p h o", o=1).broadcast_to(
                        [128, 4, HD]),
                    op=AluOpType.mult)

    # ---------------- LN helpers ----------------
    st1_a, ln1_a, st2_a, ln2_a = {}, {}, {}, {}

    def emit_rsqrt(out_ap, var_ap):
        # out = (var + eps)^-0.5 on VectorE: quake seed + 2 Newton steps.
        g = out_ap.shape[-1]
        vp = sb.tile([128, 8], F32, name="rsq_vp", tag="rsq_vp", bufs=4)
        t = sb.tile([128, 8], F32, name="rsq_t", tag="rsq_t", bufs=4)
        vps = vp[:, 0:g]; ts = t[:, 0:g]
        nc.vector.tensor_scalar_add(out=vps, in0=var_ap, scalar1=epsc)
        nc.vector.tensor_scalar(
            out=ts.bitcast(I32), in0=vps.bitcast(I32), scalar1=1,
            scalar2=None, op0=AluOpType.logical_shift_right)
        nc.vector.tensor_scalar(
            out=out_ap.bitcast(I32), in0=ts.bitcast(I32), scalar1=-1,
            scalar2=0x5f3759df, op0=AluOpType.mult, op1=AluOpType.add)
        nc.vector.tensor_scalar_mul(out=vps, in0=vps, scalar1=0.5)
        for _ in range(2):
            nc.vector.tensor_mul(out=ts, in0=out_ap, in1=out_ap)
            nc.vector.tensor_mul(out=ts, in0=ts, in1=vps)
            nc.vector.tensor_scalar(out=ts, in0=ts, scalar1=-1.0, scalar2=1.5,
                                    op0=AluOpType.mult, op1=AluOpType.add)
            nc.vector.tensor_mul(out=out_ap, in0=out_ap, in1=ts)

    def ln2grp(b, g):
        if b not in ln2_a:
            ln2_a[b] = (
                sb.tile([128, 8, 2], F32, name=f"mv8b_b{b}", tag="mv8", bufs=4),
                sb.tile([128, 8], F32, name=f"rs8b_b{b}", tag="rs8", bufs=4))
        mv8b, rs8b = ln2_a[b]
        gs = slice(g * 4, g * 4 + 4)
        for lt in range(g * 4, g * 4 + 4):
            nc.vector.bn_aggr(out=mv8b[:, lt, :], in_=st2_a[b][:, lt, :])
        emit_rsqrt(rs8b[:, gs], mv8b[:, gs, 1])

    def ln1ap(b, lt, eng=None):
        # LN1 normalize + gate for one l-tile; the 2 elementwise muls/adds
        # can run on GpSimd to split the final-tail chain across engines
        # (bn_stats stays on VectorE).
        y = y_a[b]
        mv8, rs8 = ln1_a[b]
        ee = eng or nc.vector
        if b not in st2_a:
            st2_a[b] = sb.tile([128, 8, 6], F32, name=f"st8b_b{b}", tag="st8",
                               bufs=4)
        gate2_t, bg2_t = gate_a[b][lt]
        nc.vector.tensor_scalar(
            out=y[lt], in0=y[lt], scalar1=mv8[:, lt, 0:1],
            scalar2=rs8[:, lt:lt + 1],
            op0=AluOpType.subtract, op1=AluOpType.mult)
        ee.tensor_mul(out=y[lt], in0=y[lt], in1=gate2_t)
        ee.tensor_add(out=y[lt], in0=y[lt], in1=bg2_t)
        nc.vector.bn_stats(out=st2_a[b][:, lt, :], in_=y[lt])

    def ln1grp(b, g):
        # aggregate LN1 stats for l-tiles [4g, 4g+4)
        if b not in ln1_a:
            ln1_a[b] = (
                sb.tile([128, 8, 2], F32, name=f"mv8_b{b}", tag="mv8", bufs=4),
                sb.tile([128, 8], F32, name=f"rs8_b{b}", tag="rs8", bufs=4))
        mv8, rs8 = ln1_a[b]
        gs = slice(g * 4, g * 4 + 4)
        for l2 in range(g * 4, g * 4 + 4):
            nc.vector.bn_aggr(out=mv8[:, l2, :], in_=st1_a[b][:, l2, :])
        emit_rsqrt(rs8[:, gs], mv8[:, gs, 1])

    def emit_tail_lt(b, lt, final=False):
        # after both head-groups' PV for lt: lepe transpose-add + LN1 stats.
        # Both ct transposes land in one psum tile so a single merged DVE
        # add covers all 256 channels.
        y = y_a[b]
        if b not in st1_a:
            st1_a[b] = sb.tile([128, 8, 6], F32, name=f"st8_b{b}", tag="st8",
                               bufs=4)
        if final:
            ps = pp.tile([128, 2, 128], BF16, name="ps_trf", tag="ps2",
                         bufs=2)
        else:
            ps = pp.tile([128, 2, 128], BF16, name="ps_tr", tag="ps_mm",
                         bufs=1)
        for ct in range(2):
            nc.tensor.transpose(ps[:, ct, :],
                                lepe_a[b][ct][:, lt * 128:(lt + 1) * 128],
                                ident)
        nc.vector.tensor_add(out=y[lt], in0=y[lt],
                             in1=ps.rearrange("p a b -> p (a b)"))
        nc.vector.bn_stats(out=st1_a[b][:, lt, :], in_=y[lt])

    y2T_a = {}

    def lnhalf_lt(b, lt, final=False):
        # LN2 normalize + transpose to [c, l] for one l-tile
        y = y_a[b]
        mv8b, rs8b = ln2_a[b]
        if b not in y2T_a:
            y2T_a[b] = [sb.tile([128, L], BF16, name=f"y2T_b{b}c{ct2}",
                                tag="y2T", bufs=4) for ct2 in range(2)]
        y2T = y2T_a[b]
        y2b = sb.tile([128, C], BF16, name="y2b", tag="y2b", bufs=8)
        nc.vector.tensor_scalar(
            out=y2b, in0=y[lt], scalar1=mv8b[:, lt, 0:1],
            scalar2=rs8b[:, lt:lt + 1],
            op0=AluOpType.subtract, op1=AluOpType.mult)
        for ct in range(2):
            if final:
                ps = pp.tile([128, 128], BF16, name="ps_tr2b",
                             tag="ps2", bufs=2)
            else:
                ps = pp.tile([128, 128], BF16, name="ps_tr2",
                             tag="ps_mm", bufs=1)
            nc.tensor.transpose(
                ps, y2b[:, ct * 128:(ct + 1) * 128], ident)
            dst = y2T[ct][:, lt * 128:(lt + 1) * 128]
            if final:
                nc.scalar.copy(out=dst, in_=ps)
            else:
                nc.vector.tensor_copy(out=dst, in_=ps)

    def emit_proj(b, mt, n, final=False):
        y2T = y2T_a[b]
        o_t = sb.tile([128, 512], F32, name=f"o_b{b}m{mt}n{n}", tag="osb",
                      bufs=2)
        if final:
            ps = pp.tile([128, 512], F32, name="ps_projf", tag="ps_pv", bufs=2)
        else:
            ps = pp.tile([128, 512], F32, name="ps_proj", tag="ps_mm", bufs=1)
        for kc in range(2):
            nc.tensor.matmul(
                ps, wprojT[kc][:, mt * 128:(mt + 1) * 128],
                y2T[kc][:, n * 512:(n + 1) * 512],
                start=(kc == 0), stop=(kc == 1))
        nc.vector.tensor_scalar_add(
            out=o_t, in0=ps, scalar1=bproj[:, mt:mt + 1])
        oeng = (nc.sync, nc.gpsimd, nc.scalar)[(b * 2 + mt + n) % 3]
        oeng.dma_start(
            out=io['out'][b, mt * 128:(mt + 1) * 128,
                          n * 512:(n + 1) * 512],
            in_=o_t)

    # ---------------- schedule ----------------
    def sc_phase(b, hg, fillers, pinned=None, pre=None, fstart=0):
        # 32 head-pair scores groups in (n, mt) order; each mt's 2 groups are
        # emitted adjacently (4 row-bands in flight).  The incremental PV
        # chains for half n step at mt+1 (one group behind the exp stream);
        # half 0's last step + evac overlap the start of half 1.  `pinned`
        # maps jp -> [units] run right after that jp's groups; `fillers` are
        # spread evenly over jps [fstart, 16).
        fi = 0
        pinned = pinned or {}
        if pre:
            for u in pre:
                u()
        for n in (0, 1):
            for mt in range(8):
                jp = n * 8 + mt
                for pr in (0, 1):
                    emit_scores_group(b, hg, mt, n, pr)
                if jp == 8:
                    pv_step(b, hg, 0, 7)
                    pv_evac(b, hg, 0)
                    pv_evac(b, hg, 1)
                if mt >= 1:
                    pv_step(b, hg, n, mt - 1)
                for u in pinned.get(jp, ()):
                    u()
                want = max(0, jp + 1 - fstart) * len(fillers) // (16 - fstart)
                while fi < want:
                    fillers[fi](); fi += 1
        while fi < len(fillers):
            fillers[fi](); fi += 1

    def phase_end(b, hg, scalar_norm=False):
        # close half 1: final mc step + evacuation of lt pairs 2,3.
        # Runs pinned at jp0 of the NEXT phase so its es-ACT wait hides
        # behind that phase's first score matmuls.
        pv_step(b, hg, 1, 7)
        pv_evac(b, hg, 2, scalar_norm=scalar_norm)
        pv_evac(b, hg, 3, scalar_norm=scalar_norm)

    def lepe_units_half(b, half):
        return [lambda ct=ct, seg=seg: emit_lepe_seg(b, ct, half, seg)
                for ct in range(2) for seg in range(5)]

    # ---- batch 0, head-group 0 ----
    # minimal pre-critical-path: hg0 q/k tiles (m=0 q, m=2 k), rope, vg(0,0);
    # these borrow the idle ps2 ring (big_ps) to avoid ps_mm serialization
    emit_qk_conv(0, 0, scalar_evac=True, big_ps=True)
    emit_qk_conv(0, 2, scalar_evac=True, big_ps=True)
    emit_rope(0, 0, dve_mul=True, big_ps=True, halves=(0,))
    emit_rope(0, 2, dve_mul=True, big_ps=True, halves=(0,))
    emit_rope(0, 0, dve_mul=True, big_ps=True, halves=(1,))
    emit_rope(0, 2, dve_mul=True, big_ps=True, halves=(1,))

    F0 = [lambda: emit_qk_conv(0, 1), lambda: emit_qk_conv(0, 3),
          lambda: emit_rope(0, 1), lambda: emit_rope(0, 3)]
    for ct in range(2):
        for n in range(2):
            F0.append(lambda ct=ct, n=n: emit_vcl(0, ct, n))
    F0.extend(lepe_units_half(0, 0))
    # vg(0, mt) pinned at jp=mt so chain step mc=mt (at jp=mt+1) has its vT;
    # vg(0,0) sits AFTER jp0's groups so its rhsvg-DMA wait can't stall the
    # in-order PE queue ahead of the first scores
    pin0 = {mt: [lambda mt=mt: emit_vg(0, mt)] for mt in range(8)}
    sc_phase(0, 0, F0, pinned=pin0)

    # ---- batch 0, head-group 1 ----
    # fillers: b1 q/k prep, b1 v image, b0 lepe second half; pinned: b1 vg
    # early, then b0's lt0-3 tail pipeline in per-lt/per-group units
    F0b = []
    for m in (0, 2, 1, 3):
        F0b.append(lambda m=m: emit_qk_conv(1, m))
    for t in (0, 2, 1, 3):
        F0b.append(lambda t=t: emit_rope(1, t))
    for ct in range(2):
        for n in range(2):
            F0b.append(lambda ct=ct, n=n: emit_vcl(1, ct, n))
    F0b.extend(lepe_units_half(0, 1))
    pin1 = {0: [lambda: phase_end(0, 0)]}
    for mt in range(8):
        pin1.setdefault(mt + 1, []).append(lambda mt=mt: emit_vg(1, mt))
    for i, lt in enumerate(range(4)):
        pin1.setdefault(9 + i, []).append(lambda lt=lt: emit_tail_lt(0, lt))
    pin1.setdefault(13, []).append(lambda: ln1grp(0, 0))
    pin1.setdefault(14, []).extend([lambda: ln1ap(0, 0),
                                    lambda: ln1ap(0, 1, eng=nc.gpsimd)])
    pin1.setdefault(15, []).extend([lambda: ln1ap(0, 2),
                                    lambda: ln1ap(0, 3, eng=nc.gpsimd)])
    sc_phase(0, 1, F0b, pinned=pin1)

    # ---- batch 1, head-group 0 ----
    # pinned: b0's remaining endgame spread one small unit per jp; fillers:
    # b1's lepe (its ps_lp bank no longer shared with tail transposes)
    pin2 = {0: [lambda: phase_end(0, 1), lambda: ln2grp(0, 0)],
            1: [lambda: lnhalf_lt(0, 0), lambda: lnhalf_lt(0, 1)],
            2: [lambda: lnhalf_lt(0, 2), lambda: lnhalf_lt(0, 3)],
            3: [lambda: emit_proj(0, 0, 0), lambda: emit_proj(0, 1, 0)],
            4: [lambda: emit_tail_lt(0, 4)],
            5: [lambda: emit_tail_lt(0, 5)],
            6: [lambda: emit_tail_lt(0, 6)],
            7: [lambda: emit_tail_lt(0, 7)],
            8: [lambda: ln1grp(0, 1)],
            9: [lambda: ln1ap(0, 4),
                lambda: ln1ap(0, 5, eng=nc.gpsimd)],
            10: [lambda: ln1ap(0, 6),
                 lambda: ln1ap(0, 7, eng=nc.gpsimd)],
            11: [lambda: ln2grp(0, 1)],
            12: [lambda: lnhalf_lt(0, 4), lambda: lnhalf_lt(0, 5)],
            13: [lambda: lnhalf_lt(0, 6), lambda: lnhalf_lt(0, 7)],
            14: [lambda: emit_proj(0, 0, 1), lambda: emit_proj(0, 1, 1)]}
    sc_phase(1, 0, lepe_units_half(1, 0), pinned=pin2)

    # ---- batch 1, head-group 1 ----
    pin3 = {0: [lambda: phase_end(1, 0)]}
    for i, lt in enumerate(range(4)):
        pin3.setdefault(9 + i, []).append(lambda lt=lt: emit_tail_lt(1, lt))
    pin3.setdefault(13, []).append(lambda: ln1grp(1, 0))
    pin3.setdefault(14, []).extend([lambda: ln1ap(1, 0),
                                    lambda: ln1ap(1, 1, eng=nc.gpsimd)])
    pin3.setdefault(15, []).extend([lambda: ln1ap(1, 2),
                                    lambda: ln1ap(1, 3, eng=nc.gpsimd)])
    sc_phase(1, 1, lepe_units_half(1, 1), pinned=pin3)

    # ---- final tail: interleave b1-half0's LN2/proj (independent) with
    # b1-half1's tails so PE, DVE, GpSimd and ScalarE all overlap
    phase_end(1, 1, scalar_norm=True)
    ln2grp(1, 0)
    emit_tail_lt(1, 4, final=True)
    emit_tail_lt(1, 5, final=True)
    emit_tail_lt(1, 6, final=True)
    emit_tail_lt(1, 7, final=True)
    lnhalf_lt(1, 0, final=True)
    lnhalf_lt(1, 1, final=True)
    lnhalf_lt(1, 2, final=True)
    lnhalf_lt(1, 3, final=True)
    emit_proj(1, 0, 0, final=True)
    emit_proj(1, 1, 0, final=True)
    ln1grp(1, 1)
    ln1ap(1, 4)
    ln1ap(1, 5, eng=nc.gpsimd)
    ln1ap(1, 6)
    ln1ap(1, 7, eng=nc.gpsimd)
    ln2grp(1, 1)
    lnhalf_lt(1, 4, final=True)
    lnhalf_lt(1, 5, final=True)
    # last projections in column-quarters interleaved with the lnhalf
    # units that feed them: quarter q reads y2T cols 512+256q, ready after
    # lnhalf of lt 4+2q and 5+2q
    for q in range(2):
        if q == 1:
            lnhalf_lt(1, 6, final=True)
            lnhalf_lt(1, 7, final=True)
        for mt in range(2):
            o_t = sb.tile([128, 256], F32, name=f"o_fq{mt}{q}", tag="osb",
                          bufs=2)
            ps = pp.tile([128, 256], F32, name="ps_projq", tag="ps_pv",
                         bufs=2)
            for kc in range(2):
                nc.tensor.matmul(
                    ps, wprojT[kc][:, mt * 128:(mt + 1) * 128],
                    y2T_a[1][kc][:, 512 + q * 256:512 + (q + 1) * 256],
                    start=(kc == 0), stop=(kc == 1))
            nc.scalar.activation(out=o_t, in_=ps, func=AF.Identity,
                                 bias=bproj[:, mt:mt + 1], scale=1.0)
            oeng = (nc.sync, nc.gpsimd, nc.scalar)[(mt + q) % 3]
            oeng.dma_start(
                out=io['out'][1, mt * 128:(mt + 1) * 128,
                              512 + q * 256:512 + (q + 1) * 256],
                in_=o_t)


# ----------------------------------------------------------------------
# host side
# ----------------------------------------------------------------------
def host_prep(inp):
    f32 = np.float32
    bf = lambda a: np.ascontiguousarray(a).astype(NPBF)
    p = {}
    w_qkv = np.asarray(inp['w_qkv'], f32)
    b_qkv = np.asarray(inp['b_qkv'], f32)
    # q/k weights with 4-heads-per-tile packing: head h -> tile h//4,
    # partition offset 32*(h%4); k block starts at column 256.
    wqk_pad = np.zeros((C, 512), f32)
    bqk_pad = np.zeros(512, f32)
    for h in range(NH):
        dst = (h // 4) * 128 + (h % 4) * 32
        wqk_pad[:, dst:dst + 32] = w_qkv[h * 32:(h + 1) * 32].T
        wqk_pad[:, 256 + dst:256 + dst + 32] = \
            w_qkv[256 + h * 32:256 + (h + 1) * 32].T
        bqk_pad[dst:dst + 32] = b_qkv[h * 32:(h + 1) * 32]
        bqk_pad[256 + dst:256 + dst + 32] = b_qkv[256 + h * 32:256 + (h + 1) * 32]
    p['wqkT'] = bf(wqk_pad)
    p['bqk'] = np.ascontiguousarray(bqk_pad.reshape(4, 128).T)
    p['wvT'] = bf(w_qkv[512:].T)
    p['bv'] = np.ascontiguousarray(b_qkv[512:].reshape(2, 128).T)
    s = np.asarray(inp['bn_gamma'], f32) / np.sqrt(np.float32(1.0) + f32(BN_EPS))
    wg = np.asarray(inp['w_gate'], f32) * s[:, None]
    bg = np.asarray(inp['b_gate'], f32) * s + np.asarray(inp['bn_beta'], f32)
    p['rhsvg'] = bf(np.concatenate([w_qkv[512:].T, wg.T], axis=1))
    p['bvgbc'] = bf(np.tile(np.concatenate([b_qkv[512:], bg])[None, :], (128, 1)))
    wp = np.asarray(inp['w_proj'], f32) * np.asarray(inp['ln_gamma'], f32)[None, :]
    bp = (np.asarray(inp['b_proj'], f32)
          + np.asarray(inp['w_proj'], f32) @ np.asarray(inp['ln_beta'], f32))
    p['wprojT'] = bf(wp.T)
    p['bproj'] = np.ascontiguousarray(bp.reshape(2, 128).T)
    cosl = np.asarray(inp['cos'], f32).reshape(L, HD).T
    sinl = np.asarray(inp['sin'], f32).reshape(L, HD).T
    p['cosq'] = bf(np.tile(cosl, (4, 1)))
    p['sinq'] = bf(np.tile(sinl, (4, 1)))
    R = np.zeros((128, 128), f32)
    for i in range(64):
        R[2 * i + 1, 2 * i] = -1.0
        R[2 * i, 2 * i + 1] = 1.0
    p['rotmat'] = bf(R)
    p['ident'] = bf(np.eye(128, dtype=f32))
    # diag[ct, :, tap*128:(tap+1)*128] = diag(w5[ct, :, tap])
    w5 = np.asarray(inp['w_lepe'], f32).reshape(2, 128, 25)
    dw = np.zeros((2, 128, 25 * 128), f32)
    idx = np.arange(128)
    for ct in range(2):
        for tap in range(25):
            dw[ct, idx, tap * 128 + idx] = w5[ct, :, tap]
    p['diagw'] = bf(dw)
    p['blepe'] = np.ascontiguousarray(
        np.asarray(inp['b_lepe'], f32).reshape(2, 128).T)
    # gate is computed as g*(1+tanh(g/2)) = 2*silu(g); the 0.5 is folded here
    p['g1bc'] = np.tile(0.5 * np.asarray(inp['norm_gamma'], f32)[None, :], (128, 1))
    p['b1bc'] = np.tile(0.5 * np.asarray(inp['norm_beta'], f32)[None, :], (128, 1))
    return p


_NC = None


def _get_nc():
    global _NC
    if _NC is None:
        _NC = build_program()
    return _NC


def make_in_maps(inputs):
    p = host_prep(inputs)
    x = np.asarray(inputs['x'], np.float32).reshape(B, C, L)
    in_maps = []
    for i in range(NCORES):
        m = dict(p)
        m['x2'] = np.ascontiguousarray(x[i * BPC:(i + 1) * BPC]).astype(NPBF)
        in_maps.append(m)
    return in_maps


def kernel(**inputs):
    from concourse.bass_utils import run_bass_kernel_spmd
    nc = _get_nc()
    in_maps = make_in_maps(inputs)
    res = run_bass_kernel_spmd(nc, in_maps, core_ids=list(range(NCORES)))
    outs = [np.asarray(res.results[i]['out'], np.float32).reshape(BPC, C, H, W)
            for i in range(NCORES)]
    return np.concatenate(outs, axis=0)
